# revision 11
# baseline (speedup 1.0000x reference)
"""Trainium2 Bass kernel for a 4D ConvBlock (conv3^4 -> LN -> GELU -> 1x1 conv -> residual).

Strategy (8 NeuronCores, data-parallel over T with halo 1):
  - Core t computes the full output t-slice out[:, :, t] for BOTH batch samples.
  - Partition layout: 128 SBUF partitions = (sample n)*64 + channel c.
  - conv1 is computed as 81 accumulating PE matmuls (one per 3x3x3x3 kernel
    offset) with BLOCK-DIAGONAL weights [128,128] so both samples ride one
    matmul (K=64 channels would otherwise waste half the 128-wide PE array).
  - Spatial H/W halos come from zero-padded SBUF slices (34x34 per (l) slice,
    padded on host); L halos are handled by skipping out-of-range dl offsets;
    T halos by zero-filled neighbor t-slices on edge cores.
  - Channel-wise LayerNorm stats via tiny matmuls (ones-reduce K=128->M=2 per
    sample), broadcast back with a [2->128] matmul; exact-erf GELU on ACT.
  - conv2 (1x1) is a single block-diagonal matmul; residual read straight from
    the padded input slice.
  - Matmuls run in float32r (TF32, full PE rate). The BIR verifier requires
    every matmul operand's producer to round to f32r, so matmul-feeding tiles
    are DECLARED float32r (DMA'd ones come from f32r DRAM tensors; computed
    ones are written by ACT/DVE ops that round on write). Non-matmul consumers
    read those tiles through a bitcast back to f32.
"""
import os
import sys

os.environ.setdefault("MYCRO_LOCAL_CACHE", "1")
for _p in ("/opt/trn_rl_repo",):
    if os.path.isdir(_p) and _p not in sys.path:
        sys.path.insert(0, _p)

import numpy as np

import concourse.bass as bass
import concourse.tile as tile
from concourse import bacc, mybir
from concourse import bass_utils

# float32 = exact, quarter-rate PE. float32r = TF32, full-rate PE.
MM_DTYPE = os.environ.get("MM_DTYPE", "float32r")
TRACE = os.environ.get("KERNEL_TRACE", "0") == "1"

N, C, T, L, H, W = 2, 64, 8, 8, 32, 32
P = 128
EPS = 1e-5
OFFSETS = [(dt, dl, dh, dw)
           for dt in (-1, 0, 1) for dl in (-1, 0, 1)
           for dh in (-1, 0, 1) for dw in (-1, 0, 1)]

_CACHE = {}
LAST_RESULTS = None


def _build(mm_dtype_str):
    f32 = mybir.dt.float32
    mmdt = getattr(mybir.dt, mm_dtype_str)
    AF = mybir.ActivationFunctionType

    def asf32(ap):
        return ap if ap.dtype == f32 else ap.bitcast(f32)

    nc = bacc.Bacc("TRN2", target_bir_lowering=False, debug=False,
                   enable_asserts=False, num_devices=8)
    xinp = nc.dram_tensor("xinp", [3, P, L, H + 2, W + 2], mmdt,
                          kind="ExternalInput").ap()
    w1c = nc.dram_tensor("w1c", [C, 81, C], mmdt, kind="ExternalInput").ap()
    w2bd = nc.dram_tensor("w2bd", [P, P], mmdt, kind="ExternalInput").ap()
    onesbc = nc.dram_tensor("onesbc", [P, P], mmdt, kind="ExternalInput").ap()
    params = nc.dram_tensor("params", [P, 5], f32, kind="ExternalInput").ap()
    out = nc.dram_tensor("out", [P, L, H, W], f32, kind="ExternalOutput").ap()

    with tile.TileContext(nc) as tc:
        with (
            tc.tile_pool(name="wpool", bufs=1) as wpool,
            tc.tile_pool(name="xpool", bufs=4) as xpool,
            tc.tile_pool(name="work", bufs=2) as work,
            tc.tile_pool(name="ps_acc", bufs=4, space=bass.MemorySpace.PSUM) as ps_acc,
            tc.tile_pool(name="ps_bc", bufs=1, space=bass.MemorySpace.PSUM) as ps_bc,
            tc.tile_pool(name="ps_out", bufs=2, space=bass.MemorySpace.PSUM) as ps_out,
        ):
            w1sb = []

            def emit_chunk(j):
                # Emission order = DMA queue priority: chunk j is emitted
                # right before its first consuming matmul so startup queues
                # drain the truly critical bytes first.
                assert j == len(w1sb)
                w1j = wpool.tile([P, 27, P], mmdt, name=f"w1sb{j}", tag=f"w1sb{j}")
                nc.vector.memset(w1j[0:C, :, C:P].bitcast(f32), 0.0)
                nc.vector.memset(w1j[C:P, :, 0:C].bitcast(f32), 0.0)
                nc.sync.dma_start(w1j[0:C, :, 0:C],
                                  w1c[:, 27 * j: 27 * (j + 1), :])
                nc.sync.dma_start(w1j[C:P, :, C:P],
                                  w1c[:, 27 * j: 27 * (j + 1), :])
                w1sb.append(w1j)

            xs = {}

            def load_slice(l):
                for tb in range(3):
                    xt = xpool.tile([P, H + 2, W + 2], mmdt,
                                    name=f"x{tb}_{l}", tag=f"x{tb}")
                    # two DMAs per slice -> more queues active during startup
                    nc.sync.dma_start(xt[:, 0:17, :], xinp[tb, :, l, 0:17, :])
                    nc.sync.dma_start(xt[:, 17:34, :], xinp[tb, :, l, 17:34, :])
                    xs[(tb, l)] = xt

            def process(l):
                act_os = [o for o, (dt, dl, dh, dw) in enumerate(OFFSETS)
                          if 0 <= l + dl < L]
                for half in range(2):
                    h0 = 16 * half
                    acc = ps_acc.tile([P, 16, W], f32,
                                      name=f"acc_{l}_{half}", tag="acc")
                    for i, o in enumerate(act_os):
                        dt, dl, dh, dw = OFFSETS[o]
                        while o // 27 >= len(w1sb):
                            emit_chunk(len(w1sb))
                        rhs = xs[(dt + 1, l + dl)][:, h0 + dh + 1: h0 + dh + 17,
                                                   dw + 1: dw + 33]
                        nc.tensor.matmul(acc[:], w1sb[o // 27][:, o % 27, :], rhs,
                                         start=(i == 0),
                                         stop=(i == len(act_os) - 1))
                    h = work.tile([P, 16, W], mmdt, name=f"h_{l}_{half}", tag="h")
                    nc.vector.tensor_scalar_add(h[:], acc[:], b1_ap)
                    sq = work.tile([P, 16, W], mmdt, name=f"sq_{l}_{half}", tag="sq")
                    nc.vector.tensor_mul(sq[:], asf32(h[:]), asf32(h[:]))
                    bc_mu = ps_bc.tile([P, 16, W], f32,
                                       name=f"bcmu_{l}_{half}", tag="bc_mu")
                    nc.tensor.matmul(bc_mu[:], onsb[:], h[:])
                    bc_e2 = ps_bc.tile([P, 16, W], f32,
                                       name=f"bce2_{l}_{half}", tag="bc_e2")
                    nc.tensor.matmul(bc_e2[:], onsb[:], sq[:])
                    mu_sbf = work.tile([P, 16, W], f32,
                                       name=f"musbf_{l}_{half}", tag="mu_sbf")
                    nc.vector.tensor_copy(mu_sbf[:], bc_mu[:])
                    mu2 = work.tile([P, 16, W], f32,
                                    name=f"mu2_{l}_{half}", tag="mu2")
                    nc.vector.tensor_mul(mu2[:], mu_sbf[:], mu_sbf[:])
                    var = work.tile([P, 16, W], f32,
                                    name=f"var_{l}_{half}", tag="var")
                    nc.vector.tensor_sub(var[:], bc_e2[:], mu2[:])
                    rstd = work.tile([P, 16, W], f32,
                                     name=f"rstd_{l}_{half}", tag="rstd")
                    nc.scalar.activation(rstd[:], var[:], AF.Abs_reciprocal_sqrt,
                                         bias=eps_ap, scale=1.0)
                    t1 = work.tile([P, 16, W], f32, name=f"t1_{l}_{half}", tag="t1")
                    nc.vector.tensor_sub(t1[:], asf32(h[:]), mu_sbf[:])
                    t2 = work.tile([P, 16, W], f32, name=f"t2_{l}_{half}", tag="t2")
                    nc.vector.tensor_mul(t2[:], t1[:], rstd[:])
                    g = work.tile([P, 16, W], mmdt, name=f"g_{l}_{half}", tag="g")
                    nc.scalar.activation(g[:], t2[:], AF.Gelu,
                                         bias=lnb_ap, scale=lnw_ap)
                    ps2 = ps_out.tile([P, 16, W], f32,
                                      name=f"ps2_{l}_{half}", tag="ps2")
                    nc.tensor.matmul(ps2[:], w2sb[:], g[:])
                    o1 = work.tile([P, 16, W], f32, name=f"o1_{l}_{half}", tag="o1")
                    nc.vector.tensor_scalar_add(o1[:], ps2[:], b2_ap)
                    osb = work.tile([P, 16, W], f32,
                                    name=f"osb_{l}_{half}", tag="osb")
                    nc.vector.tensor_add(osb[:], o1[:],
                                         asf32(xs[(1, l)][:, h0 + 1: h0 + 17, 1: 33]))
                    nc.sync.dma_start(out[:, l, h0: h0 + 16, :], osb[:])

            load_slice(0)
            emit_chunk(0)
            load_slice(1)
            w2sb = wpool.tile([P, P], mmdt, name="w2sb", tag="w2sb")
            nc.sync.dma_start(w2sb[:], w2bd[:])
            onsb = wpool.tile([P, P], mmdt, name="onsb", tag="onsb")
            nc.sync.dma_start(onsb[:], onesbc[:])
            psb = wpool.tile([P, 5], f32, name="psb", tag="psb")
            nc.sync.dma_start(psb[:], params[:])
            b1_ap = psb[:, 0:1]
            lnw_ap = psb[:, 1:2]
            lnb_ap = psb[:, 2:3]
            b2_ap = psb[:, 3:4]
            eps_ap = psb[:, 4:5]

            process(0)
            for l in range(2, L + 1):
                if l < L:
                    load_slice(l)
                process(l - 1)

    nc.compile()
    return nc


def _get_program():
    key = MM_DTYPE
    if key not in _CACHE:
        _CACHE[key] = _build(key)
    return _CACHE[key]


def _host_prep(x, w1, b1, ln_w, ln_b, w2, b2):
    x = np.ascontiguousarray(np.asarray(x, dtype=np.float32))
    xm = x.reshape(N * C, T, L, H, W)
    # pad H and W by 1 on each side with zeros
    xpad = np.zeros((N * C, T, L, H + 2, W + 2), np.float32)
    xpad[:, :, :, 1:H + 1, 1:W + 1] = xm
    zslice = np.zeros((N * C, L, H + 2, W + 2), np.float32)
    xins = []
    for t in range(T):
        xp = xpad[:, t - 1] if t > 0 else zslice
        xc = xpad[:, t]
        xn = xpad[:, t + 1] if t < T - 1 else zslice
        xins.append(np.ascontiguousarray(np.stack([xp, xc, xn])))

    w1c = np.ascontiguousarray(
        np.asarray(w1, dtype=np.float32).transpose(1, 2, 3, 4, 5, 0)
    ).reshape(C, 81, C)
    w2t = np.asarray(w2, dtype=np.float32).reshape(C, C).T
    w2bd = np.zeros((P, P), np.float32)
    w2bd[:C, :C] = w2t
    w2bd[C:, C:] = w2t
    onesbc = np.zeros((P, P), np.float32)
    onesbc[:C, :C] = 1.0 / C
    onesbc[C:, C:] = 1.0 / C
    params = np.zeros((P, 5), np.float32)
    params[:, 0] = np.tile(np.asarray(b1, dtype=np.float32), 2)
    params[:, 1] = np.tile(np.asarray(ln_w, dtype=np.float32), 2)
    params[:, 2] = np.tile(np.asarray(ln_b, dtype=np.float32), 2)
    params[:, 3] = np.tile(np.asarray(b2, dtype=np.float32), 2)
    params[:, 4] = EPS
    return xins, w1c, w2bd, onesbc, params


def kernel(x, w1, b1, ln_w, ln_b, w2, b2):
    global LAST_RESULTS
    xins, w1c, w2bd, onesbc, params = _host_prep(
        x, w1, b1, ln_w, ln_b, w2, b2)
    nc = _get_program()
    in_maps = [
        {"xinp": xins[t], "w1c": w1c, "w2bd": w2bd, "onesbc": onesbc,
         "params": params}
        for t in range(T)
    ]
    res = bass_utils.run_bass_kernel_spmd(
        nc, in_maps, core_ids=list(range(8)), trace=TRACE)
    LAST_RESULTS = res
    out = np.stack([res.results[t]["out"] for t in range(T)], axis=1)
    return np.ascontiguousarray(out.reshape(N, C, T, L, H, W))


# revision 12
# speedup vs baseline: 1.0218x; 1.0218x over previous
"""Trainium2 Bass kernel for a 4D ConvBlock (conv3^4 -> LN -> GELU -> 1x1 conv -> residual).

Strategy (8 NeuronCores, data-parallel over T with halo 1):
  - Core t computes the full output t-slice out[:, :, t] for BOTH batch samples.
  - Partition layout: 128 SBUF partitions = (sample n)*64 + channel c.
  - conv1 is computed as 81 accumulating PE matmuls (one per 3x3x3x3 kernel
    offset) with BLOCK-DIAGONAL weights [128,128] so both samples ride one
    matmul (K=64 channels would otherwise waste half the 128-wide PE array).
  - Spatial H/W halos come from zero-padded SBUF slices (34x34 per (l) slice,
    padded on host); L halos are handled by skipping out-of-range dl offsets;
    T halos by zero-filled neighbor t-slices on edge cores.
  - Channel-wise LayerNorm stats via tiny matmuls (ones-reduce K=128->M=2 per
    sample), broadcast back with a [2->128] matmul; exact-erf GELU on ACT.
  - conv2 (1x1) is a single block-diagonal matmul; residual read straight from
    the padded input slice.
  - Matmuls run in float32r (TF32, full PE rate). The BIR verifier requires
    every matmul operand's producer to round to f32r, so matmul-feeding tiles
    are DECLARED float32r (DMA'd ones come from f32r DRAM tensors; computed
    ones are written by ACT/DVE ops that round on write). Non-matmul consumers
    read those tiles through a bitcast back to f32.
"""
import os
import sys

os.environ.setdefault("MYCRO_LOCAL_CACHE", "1")
for _p in ("/opt/trn_rl_repo",):
    if os.path.isdir(_p) and _p not in sys.path:
        sys.path.insert(0, _p)

import numpy as np

import concourse.bass as bass
import concourse.tile as tile
from concourse import bacc, mybir
from concourse import bass_utils

# float32 = exact, quarter-rate PE. float32r = TF32, full-rate PE.
MM_DTYPE = os.environ.get("MM_DTYPE", "float32r")
TRACE = os.environ.get("KERNEL_TRACE", "0") == "1"

N, C, T, L, H, W = 2, 64, 8, 8, 32, 32
P = 128
EPS = 1e-5
OFFSETS = [(dt, dl, dh, dw)
           for dt in (-1, 0, 1) for dl in (-1, 0, 1)
           for dh in (-1, 0, 1) for dw in (-1, 0, 1)]

_CACHE = {}
LAST_RESULTS = None


def _build(mm_dtype_str):
    f32 = mybir.dt.float32
    mmdt = getattr(mybir.dt, mm_dtype_str)
    AF = mybir.ActivationFunctionType

    def asf32(ap):
        return ap if ap.dtype == f32 else ap.bitcast(f32)

    nc = bacc.Bacc("TRN2", target_bir_lowering=False, debug=False,
                   enable_asserts=False, num_devices=8)
    xinp = nc.dram_tensor("xinp", [3, P, L, H + 2, W + 2], mmdt,
                          kind="ExternalInput").ap()
    w1c = nc.dram_tensor("w1c", [C, 81, C], mmdt, kind="ExternalInput").ap()
    w2bd = nc.dram_tensor("w2bd", [P, P], mmdt, kind="ExternalInput").ap()
    onesbc = nc.dram_tensor("onesbc", [P, P], mmdt, kind="ExternalInput").ap()
    params = nc.dram_tensor("params", [P, 5], f32, kind="ExternalInput").ap()
    out = nc.dram_tensor("out", [P, L, H, W], f32, kind="ExternalOutput").ap()

    with tile.TileContext(nc) as tc:
        with (
            tc.tile_pool(name="wpool", bufs=1) as wpool,
            tc.tile_pool(name="xpool", bufs=4) as xpool,
            tc.tile_pool(name="work", bufs=2) as work,
            tc.tile_pool(name="ps_acc", bufs=4, space=bass.MemorySpace.PSUM) as ps_acc,
            tc.tile_pool(name="ps_bc", bufs=1, space=bass.MemorySpace.PSUM) as ps_bc,
            tc.tile_pool(name="ps_out", bufs=2, space=bass.MemorySpace.PSUM) as ps_out,
        ):
            w1sb = []

            def emit_chunk(j):
                # Emission order = DMA queue priority: chunk j is emitted
                # right before its first consuming matmul so startup queues
                # drain the truly critical bytes first.
                assert j == len(w1sb)
                w1j = wpool.tile([P, 27, P], mmdt, name=f"w1sb{j}", tag=f"w1sb{j}")
                nc.vector.memset(w1j[0:C, :, C:P].bitcast(f32), 0.0)
                nc.vector.memset(w1j[C:P, :, 0:C].bitcast(f32), 0.0)
                nc.sync.dma_start(w1j[0:C, :, 0:C],
                                  w1c[:, 27 * j: 27 * (j + 1), :])
                nc.sync.dma_start(w1j[C:P, :, C:P],
                                  w1c[:, 27 * j: 27 * (j + 1), :])
                w1sb.append(w1j)

            xs = {}

            def load_one(tb, l):
                xt = xpool.tile([P, H + 2, W + 2], mmdt,
                                name=f"x{tb}_{l}", tag=f"x{tb}")
                # two DMAs per slice -> more queues active during startup
                nc.sync.dma_start(xt[:, 0:17, :], xinp[tb, :, l, 0:17, :])
                nc.sync.dma_start(xt[:, 17:34, :], xinp[tb, :, l, 17:34, :])
                xs[(tb, l)] = xt

            def load_slice(l):
                for tb in range(3):
                    load_one(tb, l)

            def process(l):
                act_os = [o for o, (dt, dl, dh, dw) in enumerate(OFFSETS)
                          if 0 <= l + dl < L]
                act_insts = []
                for half in range(2):
                    h0 = 16 * half
                    acc = ps_acc.tile([P, 16, W], f32,
                                      name=f"acc_{l}_{half}", tag="acc")
                    for i, o in enumerate(act_os):
                        dt, dl, dh, dw = OFFSETS[o]
                        while o // 27 >= len(w1sb):
                            emit_chunk(len(w1sb))
                        rhs = xs[(dt + 1, l + dl)][:, h0 + dh + 1: h0 + dh + 17,
                                                   dw + 1: dw + 33]
                        nc.tensor.matmul(acc[:], w1sb[o // 27][:, o % 27, :], rhs,
                                         start=(i == 0),
                                         stop=(i == len(act_os) - 1))
                    h = work.tile([P, 16, W], mmdt, name=f"h_{l}_{half}", tag="h")
                    nc.vector.tensor_scalar_add(h[:], acc[:], b1_ap)
                    sq = work.tile([P, 16, W], mmdt, name=f"sq_{l}_{half}", tag="sq")
                    nc.vector.tensor_mul(sq[:], asf32(h[:]), asf32(h[:]))
                    bc_mu = ps_bc.tile([P, 16, W], f32,
                                       name=f"bcmu_{l}_{half}", tag="bc_mu")
                    nc.tensor.matmul(bc_mu[:], onsb[:], h[:])
                    bc_e2 = ps_bc.tile([P, 16, W], f32,
                                       name=f"bce2_{l}_{half}", tag="bc_e2")
                    nc.tensor.matmul(bc_e2[:], onsb[:], sq[:])
                    mu_sbf = work.tile([P, 16, W], f32,
                                       name=f"musbf_{l}_{half}", tag="mu_sbf")
                    nc.vector.tensor_copy(mu_sbf[:], bc_mu[:])
                    mu2 = work.tile([P, 16, W], f32,
                                    name=f"mu2_{l}_{half}", tag="mu2")
                    nc.vector.tensor_mul(mu2[:], mu_sbf[:], mu_sbf[:])
                    var = work.tile([P, 16, W], f32,
                                    name=f"var_{l}_{half}", tag="var")
                    nc.vector.tensor_sub(var[:], bc_e2[:], mu2[:])
                    rstd = work.tile([P, 16, W], f32,
                                     name=f"rstd_{l}_{half}", tag="rstd")
                    absr_i = nc.scalar.activation(rstd[:], var[:],
                                                  AF.Abs_reciprocal_sqrt,
                                                  bias=eps_ap, scale=1.0)
                    t1 = work.tile([P, 16, W], f32, name=f"t1_{l}_{half}", tag="t1")
                    nc.vector.tensor_sub(t1[:], asf32(h[:]), mu_sbf[:])
                    t2 = work.tile([P, 16, W], f32, name=f"t2_{l}_{half}", tag="t2")
                    nc.vector.tensor_mul(t2[:], t1[:], rstd[:])
                    g = work.tile([P, 16, W], mmdt, name=f"g_{l}_{half}", tag="g")
                    gelu_i = nc.scalar.activation(g[:], t2[:], AF.Gelu,
                                                  bias=lnb_ap, scale=lnw_ap)
                    act_insts.append((absr_i, gelu_i))
                    ps2 = ps_out.tile([P, 16, W], f32,
                                      name=f"ps2_{l}_{half}", tag="ps2")
                    nc.tensor.matmul(ps2[:], w2sb[:], g[:])
                    o1 = work.tile([P, 16, W], f32, name=f"o1_{l}_{half}", tag="o1")
                    nc.vector.tensor_scalar_add(o1[:], ps2[:], b2_ap)
                    osb = work.tile([P, 16, W], f32,
                                    name=f"osb_{l}_{half}", tag="osb")
                    nc.vector.tensor_add(osb[:], o1[:],
                                         asf32(xs[(1, l)][:, h0 + 1: h0 + 17, 1: 33]))
                    nc.sync.dma_start(out[:, l, h0: h0 + 16, :], osb[:])
                if len(act_insts) == 2:
                    tile.add_dep_helper(
                        act_insts[0][1].ins, act_insts[1][0].ins, sync=True,
                        reason="batch ACT funcs: absr0,absr1,gelu0,gelu1")

            # Emission order == queue-FIFO priority == matmul consumption
            # order: chunk0, then slices tb-major (dt=-1 block reads xp first).
            emit_chunk(0)
            for _tb in range(3):
                load_one(_tb, 0)
                load_one(_tb, 1)
            w2sb = wpool.tile([P, P], mmdt, name="w2sb", tag="w2sb")
            nc.sync.dma_start(w2sb[:], w2bd[:])
            onsb = wpool.tile([P, P], mmdt, name="onsb", tag="onsb")
            nc.sync.dma_start(onsb[:], onesbc[:])
            psb = wpool.tile([P, 5], f32, name="psb", tag="psb")
            nc.sync.dma_start(psb[:], params[:])
            b1_ap = psb[:, 0:1]
            lnw_ap = psb[:, 1:2]
            lnb_ap = psb[:, 2:3]
            b2_ap = psb[:, 3:4]
            eps_ap = psb[:, 4:5]

            process(0)
            for l in range(2, L + 1):
                if l < L:
                    load_slice(l)
                process(l - 1)

    nc.compile()
    return nc


def _get_program():
    key = MM_DTYPE
    if key not in _CACHE:
        _CACHE[key] = _build(key)
    return _CACHE[key]


def _host_prep(x, w1, b1, ln_w, ln_b, w2, b2):
    x = np.ascontiguousarray(np.asarray(x, dtype=np.float32))
    xm = x.reshape(N * C, T, L, H, W)
    # pad H and W by 1 on each side with zeros
    xpad = np.zeros((N * C, T, L, H + 2, W + 2), np.float32)
    xpad[:, :, :, 1:H + 1, 1:W + 1] = xm
    zslice = np.zeros((N * C, L, H + 2, W + 2), np.float32)
    xins = []
    for t in range(T):
        xp = xpad[:, t - 1] if t > 0 else zslice
        xc = xpad[:, t]
        xn = xpad[:, t + 1] if t < T - 1 else zslice
        xins.append(np.ascontiguousarray(np.stack([xp, xc, xn])))

    w1c = np.ascontiguousarray(
        np.asarray(w1, dtype=np.float32).transpose(1, 2, 3, 4, 5, 0)
    ).reshape(C, 81, C)
    w2t = np.asarray(w2, dtype=np.float32).reshape(C, C).T
    w2bd = np.zeros((P, P), np.float32)
    w2bd[:C, :C] = w2t
    w2bd[C:, C:] = w2t
    onesbc = np.zeros((P, P), np.float32)
    onesbc[:C, :C] = 1.0 / C
    onesbc[C:, C:] = 1.0 / C
    params = np.zeros((P, 5), np.float32)
    params[:, 0] = np.tile(np.asarray(b1, dtype=np.float32), 2)
    params[:, 1] = np.tile(np.asarray(ln_w, dtype=np.float32), 2)
    params[:, 2] = np.tile(np.asarray(ln_b, dtype=np.float32), 2)
    params[:, 3] = np.tile(np.asarray(b2, dtype=np.float32), 2)
    params[:, 4] = EPS
    return xins, w1c, w2bd, onesbc, params


def kernel(x, w1, b1, ln_w, ln_b, w2, b2):
    global LAST_RESULTS
    xins, w1c, w2bd, onesbc, params = _host_prep(
        x, w1, b1, ln_w, ln_b, w2, b2)
    nc = _get_program()
    in_maps = [
        {"xinp": xins[t], "w1c": w1c, "w2bd": w2bd, "onesbc": onesbc,
         "params": params}
        for t in range(T)
    ]
    res = bass_utils.run_bass_kernel_spmd(
        nc, in_maps, core_ids=list(range(8)), trace=TRACE)
    LAST_RESULTS = res
    out = np.stack([res.results[t]["out"] for t in range(T)], axis=1)
    return np.ascontiguousarray(out.reshape(N, C, T, L, H, W))
